# revision 24
# baseline (speedup 1.0000x reference)
"""Trainium2 Bass kernel for nn_DilatedAttention (B=2, L=4096, E=512, H=8, D=64,
dilation=2, window=256, causal, pre-norm transformer block with MLP).

Strategy (v2)
-------------
* 8 cores, sequence-parallel: core c owns tokens [512c, 512c+512) of both
  batches, with a 256-token K/V halo (zero-padded on core 0).
* Dilation-2 + causal + window couples only equal-parity tokens. The HOST
  packs tokens parity-major (all even tokens, then all odd) so every on-chip
  access is stride-1; the host un-packs the output. In parity space the mask
  is a causal sliding window of 128: for each 128-query block only the
  previous and the diagonal 128-key blocks matter, masked by two constant
  triangular 0/1 matrices (multiplied into the probabilities on DVE).
* Attention produces O directly feature-major: the O matmul uses V as the
  128-stationary ([128 keys, 64]) and the probabilities as moving, with PE
  column-tiling packing two heads per PSUM tile. Softmax denominators come
  from per-head one-hot rank-8 matmuls accumulated into one [8, 128] PSUM
  row-block, reciprocal on DVE, broadcast back with a constant selector
  matmul. No transposes anywhere.
* LayerNorm stats via ones[128,128] matmuls (broadcast across partitions for
  free); rstd = Exp(-0.5*Ln(var+eps)) on the Act engine so the whole kernel
  needs only the {ln,exp} and {gelu} activation tables (square/copy/identity
  are in every table set).
* All big GEMMs (QKV, out-proj, MLP) run in fp8e4 with DoubleRow perf mode
  (2 contraction tiles per pass). Weights are pre-scaled by 64 on the host
  (fp8 subnormal avoidance) and descaled in the PSUM->SBUF copies.
  Attention score/O matmuls stay bf16.
"""

import os
import sys
import types
import numpy as np
import ml_dtypes

import concourse.bass as bass
import concourse.mybir as mybir
import concourse.tile as tile
from concourse.bass_utils import run_bass_kernel_spmd


def _install_ntff_hook_shim():
    """This image's antenv lacks axon_hooks; bass_utils imports it when
    BASS_TRACE is set.  Provide the ctypes-based NTFF hook (or a None hook)
    so tracing works — and never crashes — in any environment."""
    try:
        import antenv
    except ImportError:
        return
    try:
        from antenv.axon_hooks import get_axon_ntff_profile_hook  # noqa: F401
        return  # real module present
    except ImportError:
        pass
    import ctypes
    import contextlib

    hook = None
    so_path = "/opt/axon/libaxon_pjrt.so"
    if os.path.exists(so_path):
        try:
            lib = ctypes.CDLL(so_path)
            if hasattr(lib, "axon_start_nrt_profile"):
                lib.axon_start_nrt_profile.argtypes = [
                    ctypes.POINTER(ctypes.c_int64), ctypes.c_size_t]
                lib.axon_start_nrt_profile.restype = ctypes.c_int64
                lib.axon_stop_nrt_profile.argtypes = [ctypes.c_char_p]
                lib.axon_stop_nrt_profile.restype = ctypes.c_int64

                @contextlib.contextmanager
                def _hook(output_dir, device_ids):
                    import jax
                    jax.devices()
                    if device_ids:
                        ids = (ctypes.c_int64 * len(device_ids))(*device_ids)
                        rc = lib.axon_start_nrt_profile(ids, len(device_ids))
                    else:
                        rc = lib.axon_start_nrt_profile(None, 0)
                    if rc != 0:
                        raise RuntimeError(f"axon_start_nrt_profile rc={rc}")
                    try:
                        yield
                    finally:
                        lib.axon_stop_nrt_profile(str(output_dir).encode())

                hook = _hook
        except OSError:
            hook = None

    mod = types.ModuleType("antenv.axon_hooks")
    mod.get_axon_ntff_profile_hook = lambda: hook
    mod.set_axon_ntff_profile_hook = lambda h: None
    sys.modules["antenv.axon_hooks"] = mod
    antenv.axon_hooks = mod


_install_ntff_hook_shim()

F32 = mybir.dt.float32
BF16 = mybir.dt.bfloat16
FP8 = mybir.dt.float8e4
AF = mybir.ActivationFunctionType
ALU = mybir.AluOpType
DR = mybir.MatmulPerfMode.DoubleRow

# problem constants
B, L, E, H, D = 2, 4096, 512, 8, 64
HID = 2048
EPS = 1e-5
WIN, DIL = 256, 2
N_CORES = 8
S = L // N_CORES          # tokens per core per batch (512)
HALO = WIN                # kv halo tokens (256)
NP = 2                    # parities
U = (S + HALO) // NP      # 384 packed tokens per parity (incl. 128 halo)
UQ = S // NP              # 256 core tokens per parity
QB = UQ // 128            # 2 query blocks per parity
KBL = U // 128            # 3 key blocks per parity
NT = NP * U               # 768 packed tokens incl halo
SP = S                    # 512 core tokens, parity-major flat
EC = E // 128              # 4
HC = HID // 128            # 16
HP = H // 2                # 4 head pairs

# dtype / scaling knobs
USE_FP8 = True
DT_W = FP8 if USE_FP8 else BF16
DT_A = FP8 if USE_FP8 else BF16
WS = 64.0 if USE_FP8 else 1.0       # host-side weight pre-scale
OS = 16.0                            # O output scale (via selbc)
NPDT = ml_dtypes.float8_e4m3 if USE_FP8 else ml_dtypes.bfloat16

# engine assignment knobs (tune from trace). NOTE: gpsimd (Pool) cannot
# access PSUM, and its ALU runs at ~0.4-0.6x — only SBUF work belongs there.
ENG = {
    "xbf": "scalar",     # x fp32 -> bf16 shadow (table-free Act copy)
    "xsq": "vector",     # x^2 for LN1 stats (sbuf->sbuf)
    "xsq2": "gpsimd",    # x2^2 for LN2 stats (sbuf->sbuf)
    "musq": "gpsimd",    # mu^2 (sbuf->sbuf)
    "qcopy": "vector",   # Q psum->sbuf (+descale)
    "kcopy": "vector",   # K psum->sbuf (+descale)
    "vcopy": "vector",   # V psum->sbuf (+descale)
    "final": "vector",   # O * rbc -> oT (psum reads)
    "mu": "scalar",      # ps_mu -> mu_bf (psum read, plain scale)
    "var": "vector",     # var = ps_sq/E - musq
    "x1": "vector",      # x1 sub op
    "x1m": ("vector", "vector", "gpsimd", "gpsimd"),   # x1 mult per chunk
    "x2": "vector",      # residual add
    "x21": "vector",     # ln2 normalize sub
    "x21m": ("vector", "vector", "gpsimd", "gpsimd"),  # x21 mult per chunk
    "y": "vector",       # final residual add
}


def _legalize_waits(m, max_waits=1):
    """The walrus build here accepts only one sync-wait command per lowered
    instruction; hoist extras onto same-engine NoOps placed just before."""
    for fn in m.functions:
        for blk in fn.blocks:
            new_list = []
            for ins in blk.instructions:
                si = ins.sync_info
                if si is not None and si.on_wait is not None and len(si.on_wait) > max_waits:
                    waits = list(si.on_wait)
                    extra, keep = waits[:-max_waits], waits[-max_waits:]
                    k = 0
                    while extra:
                        chunk, extra = extra[:max_waits], extra[max_waits:]
                        nop = mybir.InstNoOp(name=f"{ins.name}-wsplit{k}", ins=[], outs=[])
                        nop.engine = ins.engine
                        nop.sync_info = mybir.SyncInfo(on_wait=chunk, on_update=[])
                        new_list.append(nop)
                        k += 1
                    si.on_wait = keep
                new_list.append(ins)
            blk.instructions = new_list


def build_program(has_qk_bias: bool, has_v_bias: bool, has_out_bias: bool, has_b2: bool):
    nc = bass.Bass("TRN2", target_bir_lowering=False, debug=False)
    E2 = 2 * E
    WSI = 1.0 / WS

    def eng(site):
        return getattr(nc, ENG[site])

    # ---- DRAM I/O ----
    xp = nc.dram_tensor("xp", [B, E, NT], F32, kind="ExternalInput").ap()
    wqkv = nc.dram_tensor("wqkv", [E, 3 * E], DT_W, kind="ExternalInput").ap()
    wout = nc.dram_tensor("wout", [E, E], DT_W, kind="ExternalInput").ap()
    w1 = nc.dram_tensor("w1", [E, HID], DT_W, kind="ExternalInput").ap()
    w2 = nc.dram_tensor("w2", [HID, E], DT_W, kind="ExternalInput").ap()
    vmlp_in = nc.dram_tensor("vmlp", [HID], F32, kind="ExternalInput").ap()
    msk_in = nc.dram_tensor("msk", [128, 4 * 128], BF16, kind="ExternalInput").ap()
    ohsel_in = nc.dram_tensor("ohsel", [128, 2 * H * H], BF16, kind="ExternalInput").ap()
    selbc_in = nc.dram_tensor("selbc", [8, HP * 128], BF16, kind="ExternalInput").ap()
    if has_qk_bias:
        vqk_in = nc.dram_tensor("vqk", [2 * E], F32, kind="ExternalInput").ap()
    if has_v_bias:
        vvb_in = nc.dram_tensor("vvb", [E], F32, kind="ExternalInput").ap()
        vhalo_in = nc.dram_tensor("vhalo", [128], F32, kind="ExternalInput").ap()
    if has_out_bias:
        outb_in = nc.dram_tensor("outb", [E], F32, kind="ExternalInput").ap()
    if has_b2:
        b2_in = nc.dram_tensor("b2v", [E], F32, kind="ExternalInput").ap()
    yT = nc.dram_tensor("yT", [B, E, SP], F32, kind="ExternalOutput").ap()

    with tile.TileContext(nc) as tc:
        ctxstack = []

        def pool(name, bufs, space="SBUF"):
            p = tc.tile_pool(name=name, bufs=bufs, space=space)
            ctxstack.append(p)
            return p.__enter__()

        wpool = pool("wpool", 1)
        xpool = pool("xpool", 2)
        xbfpool = pool("xbfpool", 2)
        x1pool = pool("x1pool", 2)
        stpool = pool("stpool", 2)
        qkpool = pool("qkpool", 2)
        vpool = pool("vpool", 2)
        ptpool = pool("ptpool", 6)
        otpool = pool("otpool", 2)
        x2pool = pool("x2pool", 2)
        h2pool = pool("h2pool", 2)
        ypool = pool("ypool", 2)
        rpool = pool("rpool", 4)

        pmain = pool("pmain", 2, space="PSUM")
        psc = pool("psc", 2, space="PSUM")
        po = pool("po", 2, space="PSUM")
        pcomb = pool("pcomb", 2, space="PSUM")

        # ---- constants + tiny inputs on the gpsimd DMA queue (arrive first) ----
        msk_sb = wpool.tile([128, 4, 128], BF16)
        nc.gpsimd.dma_start(msk_sb, msk_in.rearrange("p (s q) -> p s q", s=4))
        ohsel_sb = wpool.tile([128, 2, H, H], BF16)
        nc.gpsimd.dma_start(ohsel_sb, ohsel_in.rearrange("p (k h g) -> p k h g", k=2, h=H))
        selbc_sb = wpool.tile([8, HP, 128], BF16)
        nc.gpsimd.dma_start(selbc_sb, selbc_in.rearrange("p (c q) -> p c q", c=HP))
        vmlp_sb = wpool.tile([128, HC], F32)
        nc.gpsimd.dma_start(vmlp_sb, vmlp_in.rearrange("(s p) -> p s", p=128))
        if has_qk_bias:
            vqk_sb = wpool.tile([128, 8], F32)
            nc.gpsimd.dma_start(vqk_sb, vqk_in.rearrange("(s p) -> p s", p=128))
        if has_v_bias:
            vvb_sb = wpool.tile([128, E], F32)
            nc.gpsimd.dma_start(vvb_sb, vvb_in[None, :].to_broadcast([128, E]))
            vhalo_sb = wpool.tile([128, 1], F32)
            nc.gpsimd.dma_start(vhalo_sb, vhalo_in[:, None])
        if has_out_bias:
            outb_sb = wpool.tile([128, EC], F32)
            nc.gpsimd.dma_start(outb_sb, outb_in.rearrange("(s p) -> p s", p=128))
        if has_b2:
            b2_sb = wpool.tile([128, EC], F32)
            nc.gpsimd.dma_start(b2_sb, b2_in.rearrange("(s p) -> p s", p=128))

        ones128 = wpool.tile([128, 128], BF16)
        nc.vector.memset(ones128, 1.0)
        eps_col = wpool.tile([128, 1], F32)
        nc.vector.memset(eps_col, EPS)

        # ---- big DMAs on the sync queue, ordered by first use ----
        xts = []
        for b in range(B):
            xts.append(xpool.tile([128, EC, NT], F32, tag="xt", name=f"xt{b}"))
        for c in range(EC):
            nc.sync.dma_start(xts[0][:, c], xp[0, c * 128:(c + 1) * 128, :])
        wqkv_sb = wpool.tile([128, EC, 3 * E], DT_W)
        nc.sync.dma_start(wqkv_sb, wqkv.rearrange("(c p) f -> p c f", p=128))
        for c in range(EC):
            nc.sync.dma_start(xts[1][:, c], xp[1, c * 128:(c + 1) * 128, :])
        wout_sb = wpool.tile([128, EC, E], DT_W)
        nc.sync.dma_start(wout_sb, wout.rearrange("(c p) f -> p c f", p=128))
        w1_sb = wpool.tile([128, EC, HID], DT_W)
        nc.sync.dma_start(w1_sb, w1.rearrange("(c p) f -> p c f", p=128))
        w2_sb = wpool.tile([128, HC, E], DT_W)
        nc.sync.dma_start(w2_sb, w2.rearrange("(c p) f -> p c f", p=128))

        # ================= LN stats helper =================
        def emit_stats(xstat, T):
            """xstat: [128, EC, 2, T] bf16 with slot 0 = x, slot 1 = x^2.
            Returns (mu_bf, rstd_bf) [128, T] bf16 (broadcast over partitions)."""
            ntt = T // 256
            mu_bf = stpool.tile([128, T], BF16, tag="mu", name="mu")
            rstd_bf = stpool.tile([128, T], BF16, tag="rstd", name="rstd")
            for t in range(ntt):
                t0, t1 = t * 256, (t + 1) * 256
                ps = pmain.tile([128, 2, 256], F32, tag="pmain", name="ps_stat")
                for c in range(EC):
                    nc.tensor.matmul(ps, lhsT=ones128, rhs=xstat[:, c, :, t0:t1],
                                     start=(c == 0), stop=(c == EC - 1))
                if ENG["mu"] == "scalar":
                    nc.scalar.mul(mu_bf[:, t0:t1], ps[:, 0], 1.0 / E)
                else:
                    eng("mu").tensor_scalar(mu_bf[:, t0:t1], ps[:, 0], 1.0 / E, None, ALU.mult)
                musq = stpool.tile([128, 256], F32, tag="musq", name="musq")
                if ENG["musq"] == "scalar":
                    nc.scalar.square(musq, mu_bf[:, t0:t1])
                else:
                    eng("musq").tensor_tensor(musq, mu_bf[:, t0:t1], mu_bf[:, t0:t1], ALU.mult)
                var = stpool.tile([128, 256], F32, tag="var", name="var")
                eng("var").scalar_tensor_tensor(var, ps[:, 1], 1.0 / E, musq,
                                                ALU.mult, ALU.subtract)
                lnt = stpool.tile([128, 256], F32, tag="lnt", name="lnt")
                nc.scalar.activation(lnt, var, AF.Ln, bias=eps_col)
                nc.scalar.activation(rstd_bf[:, t0:t1], lnt, AF.Exp, scale=-0.5)
            return mu_bf, rstd_bf

        # ================= LN1 + x1, both batches =================
        x1s = []
        for b in range(B):
            xt = xts[b]
            xstat = xbfpool.tile([128, EC, 2, NT], BF16, tag="xstat", name=f"xstat{b}")
            for c in range(EC):
                if ENG["xbf"] == "scalar":
                    nc.scalar.copy(xstat[:, c, 0], xt[:, c])
                else:
                    eng("xbf").tensor_copy(xstat[:, c, 0], xt[:, c])
                if ENG["xsq"] == "scalar":
                    nc.scalar.square(xstat[:, c, 1], xt[:, c])
                else:
                    eng("xsq").tensor_tensor(xstat[:, c, 1], xt[:, c], xt[:, c], ALU.mult)
            mu_bf, rstd_bf = emit_stats(xstat, NT)
            x1 = x1pool.tile([128, EC, NT], DT_A, tag="x1", name=f"x1_{b}")
            for c in range(EC):
                t1 = x1pool.tile([128, NT], BF16, tag="x1t", name="x1t")
                eng("x1").tensor_tensor(t1, xstat[:, c, 0], mu_bf, ALU.subtract)
                getattr(nc, ENG["x1m"][c]).tensor_tensor(x1[:, c], t1, rstd_bf, ALU.mult)
            x1s.append(x1)

        # ================= QKV =================
        def mm_acc(ps_slice, w_full, col0, rhs_fn, width):
            """Accumulate over the E contraction: w_full [128, EC, F] DT_W,
            columns [col0, col0+width); rhs_fn(c0, ncr) -> moving slice."""
            if USE_FP8:
                for j in range(EC // 2):
                    nc.tensor.matmul(ps_slice,
                                     lhsT=w_full[:, 2 * j:2 * j + 2, col0:col0 + width],
                                     rhs=rhs_fn(2 * j, 2),
                                     start=(j == 0), stop=(j == EC // 2 - 1),
                                     perf_mode=DR)
            else:
                for c in range(EC):
                    nc.tensor.matmul(ps_slice, lhsT=w_full[:, c, col0:col0 + width],
                                     rhs=rhs_fn(c, 1),
                                     start=(c == 0), stop=(c == EC - 1))

        def emit_qkv_closures(b):
            """Returns a list of closures, each emitting one QKV block."""
            x1 = x1s[b]
            x1v = x1.rearrange("p c (two u) -> p c two u", two=NP)
            qkT = qkpool.tile([128, 8, NT], BF16, tag="qkT", name=f"qkT{b}")
            qkTv = qkT.rearrange("p s (two u) -> p s two u", two=NP)
            vT = vpool.tile([128, KBL, NP, H, D], BF16, tag="vT", name=f"vT{b}")
            closures = []

            def k_block(fs, par):
                def go():
                    ps = pmain.tile([128, 512], F32, tag="pmain", name="ps_k")

                    def rhs(c0, ncr):
                        r = x1v[:, c0:c0 + ncr, par, :]
                        return r if ncr > 1 else r
                    mm_acc(ps[:, :U], wqkv_sb, E + fs * 128, rhs, 128)
                    dst = qkTv[:, 4 + fs, par, :]
                    if has_qk_bias:
                        eng("kcopy").tensor_scalar(dst, ps[:, :U], WSI,
                                                   vqk_sb[:, 4 + fs:5 + fs], ALU.mult, ALU.add)
                    else:
                        eng("kcopy").tensor_scalar(dst, ps[:, :U], WSI, None, ALU.mult)
                return go

            def q_block(fs):
                def go():
                    ps = pmain.tile([128, 512], F32, tag="pmain", name="ps_q")

                    def rhs(c0, ncr):
                        return x1v[:, c0:c0 + ncr, :, 128:U]
                    mm_acc(ps, wqkv_sb, fs * 128, rhs, 128)
                    dst = qkTv[:, fs, :, 128:U]
                    src = ps.rearrange("p (two u) -> p two u", two=NP)
                    if has_qk_bias:
                        eng("qcopy").tensor_scalar(dst, src, WSI,
                                                   vqk_sb[:, fs:fs + 1], ALU.mult, ALU.add)
                    else:
                        eng("qcopy").tensor_scalar(dst, src, WSI, None, ALU.mult)
                return go

            def v_block(par, kb):
                def go():
                    ps = pmain.tile([128, 512], F32, tag="pmain", name="ps_v")
                    if USE_FP8:
                        for j in range(EC // 2):
                            nc.tensor.matmul(
                                ps, lhsT=x1v[:, 2 * j:2 * j + 2, par, kb * 128:(kb + 1) * 128],
                                rhs=wqkv_sb[:, 2 * j:2 * j + 2, 2 * E:3 * E],
                                start=(j == 0), stop=(j == EC // 2 - 1), perf_mode=DR)
                    else:
                        for c in range(EC):
                            nc.tensor.matmul(
                                ps, lhsT=x1v[:, c, par, kb * 128:(kb + 1) * 128],
                                rhs=wqkv_sb[:, c, 2 * E:3 * E],
                                start=(c == 0), stop=(c == EC - 1))
                    dst = vT[:, kb, par].rearrange("p h d -> p (h d)")
                    if has_v_bias:
                        nc.vector.scalar_tensor_tensor(dst, ps, WSI, vvb_sb,
                                                       ALU.mult, ALU.add)
                        if kb == 0:
                            nc.vector.tensor_scalar(dst, dst, vhalo_sb, None, ALU.mult)
                    elif ENG["vcopy"] == "scalar":
                        nc.scalar.mul(dst, ps, WSI)
                    else:
                        eng("vcopy").tensor_scalar(dst, ps, WSI, None, ALU.mult)
                return go

            for fs in range(4):
                for par in range(NP):
                    closures.append(k_block(fs, par))
            for fs in range(4):
                closures.append(q_block(fs))
            for par in range(NP):
                for kb in range(KBL):
                    closures.append(v_block(par, kb))
            return closures, qkTv, vT

        # ================= attention =================
        def emit_att(b, qkTv, vT, oT, filler):
            oTv = oT.rearrange("p c (two u) -> p c two u", two=NP)
            fill = list(filler)
            nfill = 0

            def pop_fill(n):
                nonlocal nfill
                for _ in range(n):
                    if fill:
                        fill.pop(0)()
                        nfill += 1

            for par in range(NP):
                pcs = [pcomb.tile([128, HP, 128], F32, tag="pcomb", name=f"pc{par}_{qb}")
                       for qb in range(QB)]
                # O accum: two tiles per par, [128, hp-pair, qb, 128]
                pos = [po.tile([128, 2, QB, 128], F32, tag="po", name=f"po{par}_{g}")
                       for g in range(2)]
                pend = []
                for h in range(H):
                    rb, sl = (h % 2) * 64, h // 2
                    # --- A: scores (3 mms) + exp + mask (pool selects) ---
                    ps4 = psc.tile([128, 4, 128], F32, tag="psc", name="ps_sc")
                    kv = qkTv[rb:rb + 64, 4 + sl, par, :]
                    qv = qkTv[rb:rb + 64, sl, par, :]
                    nc.tensor.matmul(ps4[:, 0], lhsT=kv[:, 0:128],
                                     rhs=qv[:, 128:256], start=True, stop=True)
                    nc.tensor.matmul(ps4[:, 1:3], lhsT=kv[:, 128:256],
                                     rhs=qv[:, 128:U], start=True, stop=True)
                    nc.tensor.matmul(ps4[:, 3], lhsT=kv[:, 256:U],
                                     rhs=qv[:, 256:U], start=True, stop=True)
                    pt = ptpool.tile([128, 4, 128], BF16, tag="pt", name="pt")
                    nc.scalar.activation(pt, ps4, AF.Exp)
                    ptr = pt.rearrange("p (a k) q -> p k a q", k=2)
                    # prev-block slots (0, 2): keep q <= k
                    nc.gpsimd.affine_select(
                        out=ptr[:, 0], in_=ptr[:, 0], compare_op=ALU.is_ge,
                        fill=0.0, base=0, channel_multiplier=1,
                        pattern=[[0, 2], [-1, 128]])
                    # diag slots (1, 3): keep k <= q
                    nc.gpsimd.affine_select(
                        out=ptr[:, 1], in_=ptr[:, 1], compare_op=ALU.is_ge,
                        fill=0.0, base=0, channel_multiplier=-1,
                        pattern=[[0, 2], [1, 128]])

                    if len(pend) == 2:
                        pend.pop(0)()
                    pop_fill(1)

                    def b_stage(h=h, rb=rb, sl=sl, pt=pt):
                        for qb in range(QB):
                            for kb in range(2):
                                kind = 0 if (qb == 0 and kb == 0) else 1
                                nc.tensor.matmul(
                                    pcs[qb][0:8, 0, :], lhsT=ohsel_sb[:, kind, h],
                                    rhs=pt[:, 2 * qb + kb],
                                    start=(h == 0 and kb == 0), stop=(h == H - 1 and kb == 1))
                        g, hh = sl // 2, sl % 2
                        # k0 feeds qb0-diag and qb1-prev in one 256-col pass
                        nc.tensor.matmul(
                            pos[g][rb:rb + 64, hh, :, :], lhsT=vT[:, 1, par, h],
                            rhs=pt[:, 1:3], start=True, stop=False,
                            skip_group_check=True)
                        nc.tensor.matmul(
                            pos[g][rb:rb + 64, hh, 0, :], lhsT=vT[:, 0, par, h],
                            rhs=pt[:, 0], start=False, stop=True,
                            skip_group_check=True)
                        nc.tensor.matmul(
                            pos[g][rb:rb + 64, hh, 1, :], lhsT=vT[:, 2, par, h],
                            rhs=pt[:, 3], start=False, stop=True,
                            skip_group_check=True)
                    pend.append(b_stage)
                for p_ in pend:
                    p_()
                # --- C: denominators + broadcast + final scale ---
                for qb in range(QB):
                    rden = rpool.tile([8, 128], BF16, tag="rden", name="rden")
                    with nc.allow_low_precision(reason="attn denom recip in bf16"):
                        nc.vector.reciprocal(rden, pcs[qb][0:8, 0, :])
                    for hp in range(HP):
                        nc.tensor.matmul(pcs[qb][:, hp, :], lhsT=selbc_sb[:, hp, :],
                                         rhs=rden, start=True, stop=True)
                    rbc_sb = rpool.tile([128, HP, 128], BF16, tag="rbc", name="rbc")
                    nc.scalar.copy(rbc_sb, pcs[qb])
                    for g in range(2):
                        eng("final").tensor_tensor(
                            oTv[:, 2 * g:2 * g + 2, par, qb * 128:(qb + 1) * 128],
                            pos[g][:, :, qb, :], rbc_sb[:, 2 * g:2 * g + 2, :], ALU.mult)
                pop_fill(1)
            pop_fill(len(fill))

        # ================= out-proj + LN2 + MLP =================
        def emit_proj_closures(b, oT, x2):
            xt = xts[b]
            xtv = xt.rearrange("p c (two u) -> p c two u", two=NP)
            closures = []

            def proj_block(es):
                def go():
                    ps = pmain.tile([128, 512], F32, tag="pmain", name="ps_proj")

                    def rhs(c0, ncr):
                        return oT[:, c0:c0 + ncr, :]
                    mm_acc(ps, wout_sb, es * 128, rhs, 128)
                    x2v = x2[:, es, 0].rearrange("p (two u) -> p two u", two=NP)
                    psv = ps.rearrange("p (two u) -> p two u", two=NP)
                    scale = WSI / OS
                    eng("x2").scalar_tensor_tensor(x2v, psv, scale,
                                                   xtv[:, es, :, 128:U], ALU.mult, ALU.add)
                    if has_out_bias:
                        eng("x2").tensor_scalar(x2[:, es, 0], x2[:, es, 0],
                                                outb_sb[:, es:es + 1], None, ALU.add)
                return go

            for es in range(EC):
                closures.append(proj_block(es))
            return closures

        def emit_ln2_stats_closures(b, x2):
            closures = []

            def sq_block(c):
                def go():
                    eng("xsq2").tensor_tensor(x2[:, c, 1], x2[:, c, 0], x2[:, c, 0],
                                              ALU.mult)
                return go

            def stats_block():
                def go():
                    res.append(emit_stats(x2, SP))
                return go
            res = []
            for c in range(EC):
                closures.append(sq_block(c))
            closures.append(stats_block())
            return closures, res

        def emit_x21(b, x2, mu_bf, rstd_bf):
            x21 = x2pool.tile([128, EC, SP], DT_A, tag="x21", name=f"x21_{b}")
            for c in range(EC):
                t1 = x1pool.tile([128, SP], BF16, tag="x21t", name="x21t")
                eng("x21").tensor_tensor(t1, x2[:, c, 0], mu_bf, ALU.subtract)
                getattr(nc, ENG["x21m"][c]).tensor_tensor(x21[:, c], t1, rstd_bf, ALU.mult)
            return x21

        def emit_mlp1(b, x21, h2):
            for hs in range(HC):
                ps = pmain.tile([128, 512], F32, tag="pmain", name="ps_m1")

                def rhs(c0, ncr):
                    return x21[:, c0:c0 + ncr, :]
                mm_acc(ps, w1_sb, hs * 128, rhs, 128)
                nc.scalar.activation(h2[:, hs], ps, AF.Gelu,
                                     bias=vmlp_sb[:, hs:hs + 1], scale=WSI)

        def emit_mlp2(b, h2, x2):
            for es in range(EC):
                ps = pmain.tile([128, 512], F32, tag="pmain", name="ps_m2")
                if USE_FP8:
                    for k in range(HC // 2):
                        nc.tensor.matmul(ps, lhsT=w2_sb[:, 2 * k:2 * k + 2, es * 128:(es + 1) * 128],
                                         rhs=h2[:, 2 * k:2 * k + 2, :],
                                         start=(k == 0), stop=(k == HC // 2 - 1),
                                         perf_mode=DR)
                else:
                    for hc in range(HC):
                        nc.tensor.matmul(ps, lhsT=w2_sb[:, hc, es * 128:(es + 1) * 128],
                                         rhs=h2[:, hc, :],
                                         start=(hc == 0), stop=(hc == HC - 1))
                yt = ypool.tile([128, SP], F32, tag="yt", name="yt")
                eng("y").scalar_tensor_tensor(yt, ps, WSI, x2[:, es, 0], ALU.mult, ALU.add)
                if has_b2:
                    eng("y").tensor_scalar(yt, yt, b2_sb[:, es:es + 1], None, ALU.add)
                nc.sync.dma_start(yT[b, es * 128:(es + 1) * 128, :], yt)

        # ================= schedule =================
        qkv0, qkTv0, vT0 = emit_qkv_closures(0)
        for cl in qkv0:
            cl()
        qkv1, qkTv1, vT1 = emit_qkv_closures(1)

        oT0 = otpool.tile([128, EC, SP], DT_A, tag="oT", name="oT0")
        emit_att(0, qkTv0, vT0, oT0, qkv1)

        x2_0 = x2pool.tile([128, EC, 2, SP], BF16, tag="x2", name="x2_0")
        proj0 = emit_proj_closures(0, oT0, x2_0)
        ln2s0, ln2res0 = emit_ln2_stats_closures(0, x2_0)

        oT1 = otpool.tile([128, EC, SP], DT_A, tag="oT", name="oT1")
        emit_att(1, qkTv1, vT1, oT1, proj0 + ln2s0)

        mu2_0, rstd2_0 = ln2res0[0]
        x21_0 = emit_x21(0, x2_0, mu2_0, rstd2_0)

        # batch 1 out-proj + LN2 (PE work overlapping batch 0's gelu stream)
        x2_1 = x2pool.tile([128, EC, 2, SP], BF16, tag="x2", name="x2_1")
        for cl in emit_proj_closures(1, oT1, x2_1):
            cl()
        ln2s1, ln2res1 = emit_ln2_stats_closures(1, x2_1)
        for cl in ln2s1:
            cl()

        h2_0 = h2pool.tile([128, HC, SP], DT_A, tag="h2", name="h2_0")
        emit_mlp1(0, x21_0, h2_0)

        mu2_1, rstd2_1 = ln2res1[0]
        x21_1 = emit_x21(1, x2_1, mu2_1, rstd2_1)
        h2_1 = h2pool.tile([128, HC, SP], DT_A, tag="h2", name="h2_1")
        emit_mlp1(1, x21_1, h2_1)

        emit_mlp2(0, h2_0, x2_0)
        emit_mlp2(1, h2_1, x2_1)

        for p in reversed(ctxstack):
            p.__exit__(None, None, None)

    return nc


_cached = {}


def _get_program(key):
    if key not in _cached:
        nc = build_program(*key)
        _legalize_waits(nc.m)
        _cached[key] = nc
    return _cached[key]


def _prepare_core_inputs(inputs):
    """Host-side folding + parity packing + sharding."""
    x = np.asarray(inputs["x"], np.float32)
    ln1_g = np.asarray(inputs["ln1_g"], np.float32)
    ln1_b = np.asarray(inputs["ln1_b"], np.float32)
    qkv_w = np.asarray(inputs["qkv_w"], np.float32)
    qkv_b = np.asarray(inputs["qkv_b"], np.float32)
    out_w = np.asarray(inputs["out_w"], np.float32)
    out_b = np.asarray(inputs["out_b"], np.float32)
    ln2_g = np.asarray(inputs["ln2_g"], np.float32)
    ln2_b = np.asarray(inputs["ln2_b"], np.float32)
    w1 = np.asarray(inputs["w1"], np.float32)
    b1 = np.asarray(inputs["b1"], np.float32)
    w2 = np.asarray(inputs["w2"], np.float32)
    b2 = np.asarray(inputs["b2"], np.float32)

    qscale = 1.0 / np.sqrt(D)
    wqkv_eff = ln1_g[:, None] * qkv_w
    vqkv = ln1_b @ qkv_w + qkv_b
    wqkv_eff[:, :E] *= qscale
    vqkv = vqkv.copy()
    vqkv[:E] *= qscale
    w1_eff = ln2_g[:, None] * w1
    vmlp = ln2_b @ w1 + b1

    has_qk_bias = bool(np.any(vqkv[: 2 * E] != 0.0))
    has_v_bias = bool(np.any(vqkv[2 * E:] != 0.0))
    has_out_bias = bool(np.any(out_b != 0.0))
    has_b2 = bool(np.any(b2 != 0.0))
    key = (has_qk_bias, has_v_bias, has_out_bias, has_b2)

    wqkv_c = (wqkv_eff * WS).astype(NPDT)
    wout_c = (out_w * WS).astype(NPDT)
    w1_c = (w1_eff * WS).astype(NPDT)
    w2_c = (w2 * WS).astype(NPDT)

    # masks: slot 0/2 = prev-block (keep q<=k), slot 1/3 = diag (keep k<=q)
    ki = np.arange(128)[:, None]
    qi = np.arange(128)[None, :]
    mprev = (qi <= ki).astype(np.float32)
    mdiag = (ki <= qi).astype(np.float32)
    msk = np.stack([mprev, mdiag, mprev, mdiag], axis=1)     # [128, 4, 128]
    msk = msk.reshape(128, 4 * 128).astype(ml_dtypes.bfloat16)

    # selbc: [8, HP, 128]: OS where h == 2*hp + (p>=64)
    selbc = np.zeros((8, HP, 128), np.float32)
    for hp in range(HP):
        selbc[2 * hp, hp, :64] = OS
        selbc[2 * hp + 1, hp, 64:] = OS
    selbc = selbc.reshape(8, HP * 128).astype(ml_dtypes.bfloat16)

    # x: transpose + halo + parity pack: [B, E, NP, U] -> flat [B, E, NT]
    xT_full = np.ascontiguousarray(x.transpose(0, 2, 1))  # [B, E, L]
    in_maps = []
    for core in range(N_CORES):
        s = core * S
        slab = np.zeros((B, E, S + HALO), np.float32)
        lo = s - HALO
        src_lo = max(lo, 0)
        slab[:, :, src_lo - lo:] = xT_full[:, :, src_lo:s + S]
        xpk = slab.reshape(B, E, U, NP).transpose(0, 1, 3, 2)  # [B, E, NP, U]
        xpk = np.ascontiguousarray(xpk).reshape(B, E, NT)

        # ohsel: [128, 2, H, H]: kind 0 = halo-valid one-hot, kind 1 = ones one-hot
        ohsel = np.zeros((128, 2, H, H), np.float32)
        halo_valid = 0.0 if core == 0 else 1.0
        for h in range(H):
            ohsel[:, 0, h, h] = halo_valid
            ohsel[:, 1, h, h] = 1.0
        ohsel = ohsel.reshape(128, 2 * H * H).astype(ml_dtypes.bfloat16)

        im = {
            "xp": xpk,
            "wqkv": wqkv_c,
            "wout": wout_c,
            "w1": w1_c,
            "w2": w2_c,
            "vmlp": vmlp.astype(np.float32),
            "msk": msk,
            "ohsel": ohsel,
            "selbc": selbc,
        }
        if has_qk_bias:
            im["vqk"] = vqkv[: 2 * E].astype(np.float32)
        if has_v_bias:
            im["vvb"] = vqkv[2 * E:].astype(np.float32)
            im["vhalo"] = np.full(128, halo_valid, np.float32)
        if has_out_bias:
            im["outb"] = out_b.astype(np.float32)
        if has_b2:
            im["b2v"] = b2.astype(np.float32)
        in_maps.append(im)
    return key, in_maps


_last_results = None


def kernel(**inputs) -> np.ndarray:
    global _last_results
    key, in_maps = _prepare_core_inputs(inputs)
    nc = _get_program(key)
    res = run_bass_kernel_spmd(nc, in_maps, core_ids=list(range(N_CORES)))
    _last_results = res
    out = np.empty((B, L, E), np.float32)
    for core in range(N_CORES):
        yp = res.results[core]["yT"]          # [B, E, SP] parity-packed
        yu = yp.reshape(B, E, NP, UQ).transpose(0, 1, 3, 2).reshape(B, E, S)
        out[:, core * S:(core + 1) * S, :] = yu.transpose(0, 2, 1)
    return out


# revision 26
# speedup vs baseline: 1.0035x; 1.0035x over previous
"""Trainium2 Bass kernel for nn_DilatedAttention (B=2, L=4096, E=512, H=8, D=64,
dilation=2, window=256, causal, pre-norm transformer block with MLP).

Strategy (v2)
-------------
* 8 cores, sequence-parallel: core c owns tokens [512c, 512c+512) of both
  batches, with a 256-token K/V halo (zero-padded on core 0).
* Dilation-2 + causal + window couples only equal-parity tokens. The HOST
  packs tokens parity-major (all even tokens, then all odd) so every on-chip
  access is stride-1; the host un-packs the output. In parity space the mask
  is a causal sliding window of 128: for each 128-query block only the
  previous and the diagonal 128-key blocks matter, masked by two constant
  triangular 0/1 matrices (multiplied into the probabilities on DVE).
* Attention produces O directly feature-major: the O matmul uses V as the
  128-stationary ([128 keys, 64]) and the probabilities as moving, with PE
  column-tiling packing two heads per PSUM tile. Softmax denominators come
  from per-head one-hot rank-8 matmuls accumulated into one [8, 128] PSUM
  row-block, reciprocal on DVE, broadcast back with a constant selector
  matmul. No transposes anywhere.
* LayerNorm stats via ones[128,128] matmuls (broadcast across partitions for
  free); rstd = Exp(-0.5*Ln(var+eps)) on the Act engine so the whole kernel
  needs only the {ln,exp} and {gelu} activation tables (square/copy/identity
  are in every table set).
* All big GEMMs (QKV, out-proj, MLP) run in fp8e4 with DoubleRow perf mode
  (2 contraction tiles per pass). Weights are pre-scaled by 64 on the host
  (fp8 subnormal avoidance) and descaled in the PSUM->SBUF copies.
  Attention score/O matmuls stay bf16.
"""

import os
import sys
import types
import numpy as np
import ml_dtypes

import concourse.bass as bass
import concourse.mybir as mybir
import concourse.tile as tile
from concourse.bass_utils import run_bass_kernel_spmd


def _install_ntff_hook_shim():
    """This image's antenv lacks axon_hooks; bass_utils imports it when
    BASS_TRACE is set.  Provide the ctypes-based NTFF hook (or a None hook)
    so tracing works — and never crashes — in any environment."""
    try:
        import antenv
    except ImportError:
        return
    try:
        from antenv.axon_hooks import get_axon_ntff_profile_hook  # noqa: F401
        return  # real module present
    except ImportError:
        pass
    import ctypes
    import contextlib

    hook = None
    so_path = "/opt/axon/libaxon_pjrt.so"
    if os.path.exists(so_path):
        try:
            lib = ctypes.CDLL(so_path)
            if hasattr(lib, "axon_start_nrt_profile"):
                lib.axon_start_nrt_profile.argtypes = [
                    ctypes.POINTER(ctypes.c_int64), ctypes.c_size_t]
                lib.axon_start_nrt_profile.restype = ctypes.c_int64
                lib.axon_stop_nrt_profile.argtypes = [ctypes.c_char_p]
                lib.axon_stop_nrt_profile.restype = ctypes.c_int64

                @contextlib.contextmanager
                def _hook(output_dir, device_ids):
                    import jax
                    jax.devices()
                    if device_ids:
                        ids = (ctypes.c_int64 * len(device_ids))(*device_ids)
                        rc = lib.axon_start_nrt_profile(ids, len(device_ids))
                    else:
                        rc = lib.axon_start_nrt_profile(None, 0)
                    if rc != 0:
                        raise RuntimeError(f"axon_start_nrt_profile rc={rc}")
                    try:
                        yield
                    finally:
                        lib.axon_stop_nrt_profile(str(output_dir).encode())

                hook = _hook
        except OSError:
            hook = None

    mod = types.ModuleType("antenv.axon_hooks")
    mod.get_axon_ntff_profile_hook = lambda: hook
    mod.set_axon_ntff_profile_hook = lambda h: None
    sys.modules["antenv.axon_hooks"] = mod
    antenv.axon_hooks = mod


_install_ntff_hook_shim()

F32 = mybir.dt.float32
BF16 = mybir.dt.bfloat16
FP8 = mybir.dt.float8e4
AF = mybir.ActivationFunctionType
ALU = mybir.AluOpType
DR = mybir.MatmulPerfMode.DoubleRow

# problem constants
B, L, E, H, D = 2, 4096, 512, 8, 64
HID = 2048
EPS = 1e-5
WIN, DIL = 256, 2
N_CORES = 8
S = L // N_CORES          # tokens per core per batch (512)
HALO = WIN                # kv halo tokens (256)
NP = 2                    # parities
U = (S + HALO) // NP      # 384 packed tokens per parity (incl. 128 halo)
UQ = S // NP              # 256 core tokens per parity
QB = UQ // 128            # 2 query blocks per parity
KBL = U // 128            # 3 key blocks per parity
NT = NP * U               # 768 packed tokens incl halo
SP = S                    # 512 core tokens, parity-major flat
EC = E // 128              # 4
HC = HID // 128            # 16
HP = H // 2                # 4 head pairs

# dtype / scaling knobs
USE_FP8 = True
DT_W = FP8 if USE_FP8 else BF16
DT_A = FP8 if USE_FP8 else BF16
WS = 64.0 if USE_FP8 else 1.0       # host-side weight pre-scale
OS = 16.0                            # O output scale (via selbc)
NPDT = ml_dtypes.float8_e4m3 if USE_FP8 else ml_dtypes.bfloat16

# engine assignment knobs (tune from trace). NOTE: gpsimd (Pool) cannot
# access PSUM, and its ALU runs at ~0.4-0.6x — only SBUF work belongs there.
ENG = {
    "xbf": "scalar",     # x fp32 -> bf16 shadow (table-free Act copy)
    "xsq": "vector",     # x^2 for LN1 stats (sbuf->sbuf)
    "xsq2": "gpsimd",    # x2^2 for LN2 stats (sbuf->sbuf)
    "musq": "gpsimd",    # mu^2 (sbuf->sbuf)
    "qcopy": "vector",   # Q psum->sbuf (+descale)
    "kcopy": "vector",   # K psum->sbuf (+descale)
    "vcopy": "vector",   # V psum->sbuf (+descale)
    "final": "vector",   # O * rbc -> oT (psum reads)
    "mu": "scalar",      # ps_mu -> mu_bf (psum read, plain scale)
    "var": "vector",     # var = ps_sq/E - musq
    "x1": "vector",      # x1 sub op
    "x1m": ("vector", "vector", "gpsimd", "gpsimd"),   # x1 mult per chunk
    "x2": "vector",      # residual add
    "x21": "vector",     # ln2 normalize sub
    "x21m": ("vector", "vector", "gpsimd", "gpsimd"),  # x21 mult per chunk
    "y": "vector",       # final residual add
}


def _legalize_waits(m, max_waits=1):
    """The walrus build here accepts only one sync-wait command per lowered
    instruction; hoist extras onto same-engine NoOps placed just before."""
    for fn in m.functions:
        for blk in fn.blocks:
            new_list = []
            for ins in blk.instructions:
                si = ins.sync_info
                if si is not None and si.on_wait is not None and len(si.on_wait) > max_waits:
                    waits = list(si.on_wait)
                    extra, keep = waits[:-max_waits], waits[-max_waits:]
                    k = 0
                    while extra:
                        chunk, extra = extra[:max_waits], extra[max_waits:]
                        nop = mybir.InstNoOp(name=f"{ins.name}-wsplit{k}", ins=[], outs=[])
                        nop.engine = ins.engine
                        nop.sync_info = mybir.SyncInfo(on_wait=chunk, on_update=[])
                        new_list.append(nop)
                        k += 1
                    si.on_wait = keep
                new_list.append(ins)
            blk.instructions = new_list


def build_program(has_qk_bias: bool, has_v_bias: bool, has_out_bias: bool, has_b2: bool):
    nc = bass.Bass("TRN2", target_bir_lowering=False, debug=False)
    E2 = 2 * E
    WSI = 1.0 / WS

    def eng(site):
        return getattr(nc, ENG[site])

    # ---- DRAM I/O ----
    xp = nc.dram_tensor("xp", [B, E, NT], F32, kind="ExternalInput").ap()
    wqkv = nc.dram_tensor("wqkv", [E, 3 * E], DT_W, kind="ExternalInput").ap()
    wout = nc.dram_tensor("wout", [E, E], DT_W, kind="ExternalInput").ap()
    w1 = nc.dram_tensor("w1", [E, HID], DT_W, kind="ExternalInput").ap()
    w2 = nc.dram_tensor("w2", [HID, E], DT_W, kind="ExternalInput").ap()
    vmlp_in = nc.dram_tensor("vmlp", [HID], F32, kind="ExternalInput").ap()
    msk_in = nc.dram_tensor("msk", [128, 4 * 128], BF16, kind="ExternalInput").ap()
    ohsel_in = nc.dram_tensor("ohsel", [128, 2 * H * H], BF16, kind="ExternalInput").ap()
    selbc_in = nc.dram_tensor("selbc", [8, HP * 128], BF16, kind="ExternalInput").ap()
    if has_qk_bias:
        vqk_in = nc.dram_tensor("vqk", [2 * E], F32, kind="ExternalInput").ap()
    if has_v_bias:
        vvb_in = nc.dram_tensor("vvb", [E], F32, kind="ExternalInput").ap()
        vhalo_in = nc.dram_tensor("vhalo", [128], F32, kind="ExternalInput").ap()
    if has_out_bias:
        outb_in = nc.dram_tensor("outb", [E], F32, kind="ExternalInput").ap()
    if has_b2:
        b2_in = nc.dram_tensor("b2v", [E], F32, kind="ExternalInput").ap()
    yT = nc.dram_tensor("yT", [B, E, SP], F32, kind="ExternalOutput").ap()

    with tile.TileContext(nc) as tc:
        ctxstack = []

        def pool(name, bufs, space="SBUF"):
            p = tc.tile_pool(name=name, bufs=bufs, space=space)
            ctxstack.append(p)
            return p.__enter__()

        wpool = pool("wpool", 1)
        xpool = pool("xpool", 2)
        xbfpool = pool("xbfpool", 2)
        x1pool = pool("x1pool", 2)
        stpool = pool("stpool", 2)
        qkpool = pool("qkpool", 2)
        vpool = pool("vpool", 2)
        ptpool = pool("ptpool", 6)
        otpool = pool("otpool", 2)
        x2pool = pool("x2pool", 2)
        h2pool = pool("h2pool", 2)
        ypool = pool("ypool", 2)
        rpool = pool("rpool", 4)

        pmain = pool("pmain", 2, space="PSUM")
        psc = pool("psc", 2, space="PSUM")
        po = pool("po", 2, space="PSUM")
        pcomb = pool("pcomb", 2, space="PSUM")

        # ---- constants + tiny inputs on the gpsimd DMA queue (arrive first) ----
        msk_sb = wpool.tile([128, 4, 128], BF16)
        nc.gpsimd.dma_start(msk_sb, msk_in.rearrange("p (s q) -> p s q", s=4))
        ohsel_sb = wpool.tile([128, 2, H, H], BF16)
        nc.gpsimd.dma_start(ohsel_sb, ohsel_in.rearrange("p (k h g) -> p k h g", k=2, h=H))
        selbc_sb = wpool.tile([8, HP, 128], BF16)
        nc.gpsimd.dma_start(selbc_sb, selbc_in.rearrange("p (c q) -> p c q", c=HP))
        vmlp_sb = wpool.tile([128, HC], F32)
        nc.gpsimd.dma_start(vmlp_sb, vmlp_in.rearrange("(s p) -> p s", p=128))
        if has_qk_bias:
            vqk_sb = wpool.tile([128, 8], F32)
            nc.gpsimd.dma_start(vqk_sb, vqk_in.rearrange("(s p) -> p s", p=128))
        if has_v_bias:
            vvb_sb = wpool.tile([128, E], F32)
            nc.gpsimd.dma_start(vvb_sb, vvb_in[None, :].to_broadcast([128, E]))
            vhalo_sb = wpool.tile([128, 1], F32)
            nc.gpsimd.dma_start(vhalo_sb, vhalo_in[:, None])
        if has_out_bias:
            outb_sb = wpool.tile([128, EC], F32)
            nc.gpsimd.dma_start(outb_sb, outb_in.rearrange("(s p) -> p s", p=128))
        if has_b2:
            b2_sb = wpool.tile([128, EC], F32)
            nc.gpsimd.dma_start(b2_sb, b2_in.rearrange("(s p) -> p s", p=128))

        ones128 = wpool.tile([128, 128], BF16)
        nc.vector.memset(ones128, 1.0)
        eps_col = wpool.tile([128, 1], F32)
        nc.vector.memset(eps_col, EPS)

        # ---- big DMAs on the sync queue, ordered by first use ----
        xts = []
        for b in range(B):
            xts.append(xpool.tile([128, EC, NT], F32, tag="xt", name=f"xt{b}"))
        for c in range(EC):
            nc.sync.dma_start(xts[0][:, c], xp[0, c * 128:(c + 1) * 128, :])
        for c in range(EC):
            nc.sync.dma_start(xts[1][:, c], xp[1, c * 128:(c + 1) * 128, :])
        wqkv_sb = wpool.tile([128, EC, 3 * E], DT_W)
        nc.sync.dma_start(wqkv_sb, wqkv.rearrange("(c p) f -> p c f", p=128))
        wout_sb = wpool.tile([128, EC, E], DT_W)
        nc.sync.dma_start(wout_sb, wout.rearrange("(c p) f -> p c f", p=128))
        w1_sb = wpool.tile([128, EC, HID], DT_W)
        nc.sync.dma_start(w1_sb, w1.rearrange("(c p) f -> p c f", p=128))
        w2_sb = wpool.tile([128, HC, E], DT_W)
        nc.sync.dma_start(w2_sb, w2.rearrange("(c p) f -> p c f", p=128))

        # ================= LN stats helper =================
        def emit_stats(xstat, T):
            """xstat: [128, EC, 2, T] bf16 with slot 0 = x, slot 1 = x^2.
            Returns (mu_bf, rstd_bf) [128, T] bf16 (broadcast over partitions)."""
            ntt = T // 256
            mu_bf = stpool.tile([128, T], BF16, tag="mu", name="mu")
            rstd_bf = stpool.tile([128, T], BF16, tag="rstd", name="rstd")
            for t in range(ntt):
                t0, t1 = t * 256, (t + 1) * 256
                ps = pmain.tile([128, 2, 256], F32, tag="pmain", name="ps_stat")
                for c in range(EC):
                    nc.tensor.matmul(ps, lhsT=ones128, rhs=xstat[:, c, :, t0:t1],
                                     start=(c == 0), stop=(c == EC - 1))
                if ENG["mu"] == "scalar":
                    nc.scalar.mul(mu_bf[:, t0:t1], ps[:, 0], 1.0 / E)
                else:
                    eng("mu").tensor_scalar(mu_bf[:, t0:t1], ps[:, 0], 1.0 / E, None, ALU.mult)
                musq = stpool.tile([128, 256], F32, tag="musq", name="musq")
                if ENG["musq"] == "scalar":
                    nc.scalar.square(musq, mu_bf[:, t0:t1])
                else:
                    eng("musq").tensor_tensor(musq, mu_bf[:, t0:t1], mu_bf[:, t0:t1], ALU.mult)
                var = stpool.tile([128, 256], F32, tag="var", name="var")
                eng("var").scalar_tensor_tensor(var, ps[:, 1], 1.0 / E, musq,
                                                ALU.mult, ALU.subtract)
                lnt = stpool.tile([128, 256], F32, tag="lnt", name="lnt")
                nc.scalar.activation(lnt, var, AF.Ln, bias=eps_col)
                nc.scalar.activation(rstd_bf[:, t0:t1], lnt, AF.Exp, scale=-0.5)
            return mu_bf, rstd_bf

        # ================= LN1 + x1, both batches =================
        x1s = []
        for b in range(B):
            xt = xts[b]
            xstat = xbfpool.tile([128, EC, 2, NT], BF16, tag="xstat", name=f"xstat{b}")
            for c in range(EC):
                if c % 2 == 0:
                    nc.scalar.copy(xstat[:, c, 0], xt[:, c])
                else:
                    nc.vector.tensor_copy(xstat[:, c, 0], xt[:, c])
                if c % 2 == 0:
                    nc.vector.tensor_tensor(xstat[:, c, 1], xt[:, c], xt[:, c], ALU.mult)
                else:
                    nc.scalar.square(xstat[:, c, 1], xt[:, c])
            mu_bf, rstd_bf = emit_stats(xstat, NT)
            x1 = x1pool.tile([128, EC, NT], DT_A, tag="x1", name=f"x1_{b}")
            for c in range(EC):
                t1 = x1pool.tile([128, NT], BF16, tag="x1t", name="x1t")
                eng("x1").tensor_tensor(t1, xstat[:, c, 0], mu_bf, ALU.subtract)
                getattr(nc, ENG["x1m"][c]).tensor_tensor(x1[:, c], t1, rstd_bf, ALU.mult)
            x1s.append(x1)

        # ================= QKV =================
        def mm_acc(ps_slice, w_full, col0, rhs_fn, width):
            """Accumulate over the E contraction: w_full [128, EC, F] DT_W,
            columns [col0, col0+width); rhs_fn(c0, ncr) -> moving slice."""
            if USE_FP8:
                for j in range(EC // 2):
                    nc.tensor.matmul(ps_slice,
                                     lhsT=w_full[:, 2 * j:2 * j + 2, col0:col0 + width],
                                     rhs=rhs_fn(2 * j, 2),
                                     start=(j == 0), stop=(j == EC // 2 - 1),
                                     perf_mode=DR)
            else:
                for c in range(EC):
                    nc.tensor.matmul(ps_slice, lhsT=w_full[:, c, col0:col0 + width],
                                     rhs=rhs_fn(c, 1),
                                     start=(c == 0), stop=(c == EC - 1))

        def emit_qkv_closures(b):
            """Returns a list of closures, each emitting one QKV block."""
            x1 = x1s[b]
            x1v = x1.rearrange("p c (two u) -> p c two u", two=NP)
            qkT = qkpool.tile([128, 8, NT], BF16, tag="qkT", name=f"qkT{b}")
            qkTv = qkT.rearrange("p s (two u) -> p s two u", two=NP)
            vT = vpool.tile([128, KBL, NP, H, D], BF16, tag="vT", name=f"vT{b}")
            closures = []

            def k_block(fs, par):
                def go():
                    ps = pmain.tile([128, 512], F32, tag="pmain", name="ps_k")

                    def rhs(c0, ncr):
                        r = x1v[:, c0:c0 + ncr, par, :]
                        return r if ncr > 1 else r
                    mm_acc(ps[:, :U], wqkv_sb, E + fs * 128, rhs, 128)
                    dst = qkTv[:, 4 + fs, par, :]
                    if has_qk_bias:
                        eng("kcopy").tensor_scalar(dst, ps[:, :U], WSI,
                                                   vqk_sb[:, 4 + fs:5 + fs], ALU.mult, ALU.add)
                    else:
                        eng("kcopy").tensor_scalar(dst, ps[:, :U], WSI, None, ALU.mult)
                return go

            def q_block(fs):
                def go():
                    ps = pmain.tile([128, 512], F32, tag="pmain", name="ps_q")

                    def rhs(c0, ncr):
                        return x1v[:, c0:c0 + ncr, :, 128:U]
                    mm_acc(ps, wqkv_sb, fs * 128, rhs, 128)
                    dst = qkTv[:, fs, :, 128:U]
                    src = ps.rearrange("p (two u) -> p two u", two=NP)
                    if has_qk_bias:
                        eng("qcopy").tensor_scalar(dst, src, WSI,
                                                   vqk_sb[:, fs:fs + 1], ALU.mult, ALU.add)
                    else:
                        eng("qcopy").tensor_scalar(dst, src, WSI, None, ALU.mult)
                return go

            def v_block(par, kb):
                def go():
                    ps = pmain.tile([128, 512], F32, tag="pmain", name="ps_v")
                    if USE_FP8:
                        for j in range(EC // 2):
                            nc.tensor.matmul(
                                ps, lhsT=x1v[:, 2 * j:2 * j + 2, par, kb * 128:(kb + 1) * 128],
                                rhs=wqkv_sb[:, 2 * j:2 * j + 2, 2 * E:3 * E],
                                start=(j == 0), stop=(j == EC // 2 - 1), perf_mode=DR)
                    else:
                        for c in range(EC):
                            nc.tensor.matmul(
                                ps, lhsT=x1v[:, c, par, kb * 128:(kb + 1) * 128],
                                rhs=wqkv_sb[:, c, 2 * E:3 * E],
                                start=(c == 0), stop=(c == EC - 1))
                    dst = vT[:, kb, par].rearrange("p h d -> p (h d)")
                    if has_v_bias:
                        nc.vector.scalar_tensor_tensor(dst, ps, WSI, vvb_sb,
                                                       ALU.mult, ALU.add)
                        if kb == 0:
                            nc.vector.tensor_scalar(dst, dst, vhalo_sb, None, ALU.mult)
                    elif ENG["vcopy"] == "scalar":
                        nc.scalar.mul(dst, ps, WSI)
                    else:
                        eng("vcopy").tensor_scalar(dst, ps, WSI, None, ALU.mult)
                return go

            for fs in range(4):
                for par in range(NP):
                    closures.append(k_block(fs, par))
            for fs in range(4):
                closures.append(q_block(fs))
            for par in range(NP):
                for kb in range(KBL):
                    closures.append(v_block(par, kb))
            return closures, qkTv, vT

        # ================= attention =================
        def emit_att(b, qkTv, vT, oT, filler):
            oTv = oT.rearrange("p c (two u) -> p c two u", two=NP)
            fill = list(filler)
            nfill = 0

            def pop_fill(n):
                nonlocal nfill
                for _ in range(n):
                    if fill:
                        fill.pop(0)()
                        nfill += 1

            for par in range(NP):
                pcs = [pcomb.tile([128, HP, 128], F32, tag="pcomb", name=f"pc{par}_{qb}")
                       for qb in range(QB)]
                # O accum: two tiles per par, [128, hp-pair, qb, 128]
                pos = [po.tile([128, 2, QB, 128], F32, tag="po", name=f"po{par}_{g}")
                       for g in range(2)]
                pend = []
                for h in range(H):
                    rb, sl = (h % 2) * 64, h // 2
                    # --- A: scores (3 mms) + exp + mask (pool selects) ---
                    ps4 = psc.tile([128, 4, 128], F32, tag="psc", name="ps_sc")
                    kv = qkTv[rb:rb + 64, 4 + sl, par, :]
                    qv = qkTv[rb:rb + 64, sl, par, :]
                    nc.tensor.matmul(ps4[:, 0], lhsT=kv[:, 0:128],
                                     rhs=qv[:, 128:256], start=True, stop=True)
                    nc.tensor.matmul(ps4[:, 1:3], lhsT=kv[:, 128:256],
                                     rhs=qv[:, 128:U], start=True, stop=True)
                    nc.tensor.matmul(ps4[:, 3], lhsT=kv[:, 256:U],
                                     rhs=qv[:, 256:U], start=True, stop=True)
                    pt = ptpool.tile([128, 4, 128], BF16, tag="pt", name="pt")
                    nc.scalar.activation(pt, ps4, AF.Exp)
                    ptr = pt.rearrange("p (a k) q -> p k a q", k=2)
                    # prev-block slots (0, 2): keep q <= k
                    nc.gpsimd.affine_select(
                        out=ptr[:, 0], in_=ptr[:, 0], compare_op=ALU.is_ge,
                        fill=0.0, base=0, channel_multiplier=1,
                        pattern=[[0, 2], [-1, 128]])
                    # diag slots (1, 3): keep k <= q
                    nc.gpsimd.affine_select(
                        out=ptr[:, 1], in_=ptr[:, 1], compare_op=ALU.is_ge,
                        fill=0.0, base=0, channel_multiplier=-1,
                        pattern=[[0, 2], [1, 128]])

                    if len(pend) == 2:
                        pend.pop(0)()
                    pop_fill(1)

                    def b_stage(h=h, rb=rb, sl=sl, pt=pt):
                        for qb in range(QB):
                            for kb in range(2):
                                kind = 0 if (qb == 0 and kb == 0) else 1
                                nc.tensor.matmul(
                                    pcs[qb][0:8, 0, :], lhsT=ohsel_sb[:, kind, h],
                                    rhs=pt[:, 2 * qb + kb],
                                    start=(h == 0 and kb == 0), stop=(h == H - 1 and kb == 1))
                        g, hh = sl // 2, sl % 2
                        # k0 feeds qb0-diag and qb1-prev in one 256-col pass
                        nc.tensor.matmul(
                            pos[g][rb:rb + 64, hh, :, :], lhsT=vT[:, 1, par, h],
                            rhs=pt[:, 1:3], start=True, stop=False,
                            skip_group_check=True)
                        nc.tensor.matmul(
                            pos[g][rb:rb + 64, hh, 0, :], lhsT=vT[:, 0, par, h],
                            rhs=pt[:, 0], start=False, stop=True,
                            skip_group_check=True)
                        nc.tensor.matmul(
                            pos[g][rb:rb + 64, hh, 1, :], lhsT=vT[:, 2, par, h],
                            rhs=pt[:, 3], start=False, stop=True,
                            skip_group_check=True)
                    pend.append(b_stage)
                for p_ in pend:
                    p_()
                # --- C: denominators + broadcast + final scale ---
                for qb in range(QB):
                    rden = rpool.tile([8, 128], BF16, tag="rden", name="rden")
                    with nc.allow_low_precision(reason="attn denom recip in bf16"):
                        nc.vector.reciprocal(rden, pcs[qb][0:8, 0, :])
                    for hp in range(HP):
                        nc.tensor.matmul(pcs[qb][:, hp, :], lhsT=selbc_sb[:, hp, :],
                                         rhs=rden, start=True, stop=True)
                    rbc_sb = rpool.tile([128, HP, 128], BF16, tag="rbc", name="rbc")
                    nc.scalar.copy(rbc_sb, pcs[qb])
                    for g in range(2):
                        eng("final").tensor_tensor(
                            oTv[:, 2 * g:2 * g + 2, par, qb * 128:(qb + 1) * 128],
                            pos[g][:, :, qb, :], rbc_sb[:, 2 * g:2 * g + 2, :], ALU.mult)
                pop_fill(1)
            pop_fill(len(fill))

        # ================= out-proj + LN2 + MLP =================
        def emit_proj_closures(b, oT, x2):
            xt = xts[b]
            xtv = xt.rearrange("p c (two u) -> p c two u", two=NP)
            closures = []

            def proj_block(es):
                def go():
                    ps = pmain.tile([128, 512], F32, tag="pmain", name="ps_proj")

                    def rhs(c0, ncr):
                        return oT[:, c0:c0 + ncr, :]
                    mm_acc(ps, wout_sb, es * 128, rhs, 128)
                    x2v = x2[:, es, 0].rearrange("p (two u) -> p two u", two=NP)
                    psv = ps.rearrange("p (two u) -> p two u", two=NP)
                    scale = WSI / OS
                    eng("x2").scalar_tensor_tensor(x2v, psv, scale,
                                                   xtv[:, es, :, 128:U], ALU.mult, ALU.add)
                    if has_out_bias:
                        eng("x2").tensor_scalar(x2[:, es, 0], x2[:, es, 0],
                                                outb_sb[:, es:es + 1], None, ALU.add)
                return go

            for es in range(EC):
                closures.append(proj_block(es))
            return closures

        def emit_ln2_stats_closures(b, x2):
            closures = []

            def sq_block(c):
                def go():
                    eng("xsq2").tensor_tensor(x2[:, c, 1], x2[:, c, 0], x2[:, c, 0],
                                              ALU.mult)
                return go

            def stats_block():
                def go():
                    res.append(emit_stats(x2, SP))
                return go
            res = []
            for c in range(EC):
                closures.append(sq_block(c))
            closures.append(stats_block())
            return closures, res

        def emit_x21(b, x2, mu_bf, rstd_bf):
            x21 = x2pool.tile([128, EC, SP], DT_A, tag="x21", name=f"x21_{b}")
            for c in range(EC):
                t1 = x1pool.tile([128, SP], BF16, tag="x21t", name="x21t")
                eng("x21").tensor_tensor(t1, x2[:, c, 0], mu_bf, ALU.subtract)
                getattr(nc, ENG["x21m"][c]).tensor_tensor(x21[:, c], t1, rstd_bf, ALU.mult)
            return x21

        def emit_mlp1(b, x21, h2):
            for hs in range(HC):
                ps = pmain.tile([128, 512], F32, tag="pmain", name="ps_m1")

                def rhs(c0, ncr):
                    return x21[:, c0:c0 + ncr, :]
                mm_acc(ps, w1_sb, hs * 128, rhs, 128)
                nc.scalar.activation(h2[:, hs], ps, AF.Gelu,
                                     bias=vmlp_sb[:, hs:hs + 1], scale=WSI)

        def emit_mlp2(b, h2, x2):
            for es in range(EC):
                ps = pmain.tile([128, 512], F32, tag="pmain", name="ps_m2")
                if USE_FP8:
                    for k in range(HC // 2):
                        nc.tensor.matmul(ps, lhsT=w2_sb[:, 2 * k:2 * k + 2, es * 128:(es + 1) * 128],
                                         rhs=h2[:, 2 * k:2 * k + 2, :],
                                         start=(k == 0), stop=(k == HC // 2 - 1),
                                         perf_mode=DR)
                else:
                    for hc in range(HC):
                        nc.tensor.matmul(ps, lhsT=w2_sb[:, hc, es * 128:(es + 1) * 128],
                                         rhs=h2[:, hc, :],
                                         start=(hc == 0), stop=(hc == HC - 1))
                yt = ypool.tile([128, SP], F32, tag="yt", name="yt")
                eng("y").scalar_tensor_tensor(yt, ps, WSI, x2[:, es, 0], ALU.mult, ALU.add)
                if has_b2:
                    eng("y").tensor_scalar(yt, yt, b2_sb[:, es:es + 1], None, ALU.add)
                nc.sync.dma_start(yT[b, es * 128:(es + 1) * 128, :], yt)

        # ================= schedule =================
        qkv0, qkTv0, vT0 = emit_qkv_closures(0)
        for cl in qkv0:
            cl()
        qkv1, qkTv1, vT1 = emit_qkv_closures(1)

        oT0 = otpool.tile([128, EC, SP], DT_A, tag="oT", name="oT0")
        emit_att(0, qkTv0, vT0, oT0, qkv1)

        x2_0 = x2pool.tile([128, EC, 2, SP], BF16, tag="x2", name="x2_0")
        proj0 = emit_proj_closures(0, oT0, x2_0)
        ln2s0, ln2res0 = emit_ln2_stats_closures(0, x2_0)

        oT1 = otpool.tile([128, EC, SP], DT_A, tag="oT", name="oT1")
        emit_att(1, qkTv1, vT1, oT1, proj0 + ln2s0)

        mu2_0, rstd2_0 = ln2res0[0]
        x21_0 = emit_x21(0, x2_0, mu2_0, rstd2_0)

        # batch 1 out-proj + LN2 (PE work overlapping batch 0's gelu stream)
        x2_1 = x2pool.tile([128, EC, 2, SP], BF16, tag="x2", name="x2_1")
        for cl in emit_proj_closures(1, oT1, x2_1):
            cl()
        ln2s1, ln2res1 = emit_ln2_stats_closures(1, x2_1)
        for cl in ln2s1:
            cl()

        h2_0 = h2pool.tile([128, HC, SP], DT_A, tag="h2", name="h2_0")
        emit_mlp1(0, x21_0, h2_0)

        mu2_1, rstd2_1 = ln2res1[0]
        x21_1 = emit_x21(1, x2_1, mu2_1, rstd2_1)
        h2_1 = h2pool.tile([128, HC, SP], DT_A, tag="h2", name="h2_1")
        emit_mlp1(1, x21_1, h2_1)

        emit_mlp2(0, h2_0, x2_0)
        emit_mlp2(1, h2_1, x2_1)

        for p in reversed(ctxstack):
            p.__exit__(None, None, None)

    return nc


_cached = {}


def _get_program(key):
    if key not in _cached:
        nc = build_program(*key)
        _legalize_waits(nc.m)
        _cached[key] = nc
    return _cached[key]


def _prepare_core_inputs(inputs):
    """Host-side folding + parity packing + sharding."""
    x = np.asarray(inputs["x"], np.float32)
    ln1_g = np.asarray(inputs["ln1_g"], np.float32)
    ln1_b = np.asarray(inputs["ln1_b"], np.float32)
    qkv_w = np.asarray(inputs["qkv_w"], np.float32)
    qkv_b = np.asarray(inputs["qkv_b"], np.float32)
    out_w = np.asarray(inputs["out_w"], np.float32)
    out_b = np.asarray(inputs["out_b"], np.float32)
    ln2_g = np.asarray(inputs["ln2_g"], np.float32)
    ln2_b = np.asarray(inputs["ln2_b"], np.float32)
    w1 = np.asarray(inputs["w1"], np.float32)
    b1 = np.asarray(inputs["b1"], np.float32)
    w2 = np.asarray(inputs["w2"], np.float32)
    b2 = np.asarray(inputs["b2"], np.float32)

    qscale = 1.0 / np.sqrt(D)
    wqkv_eff = ln1_g[:, None] * qkv_w
    vqkv = ln1_b @ qkv_w + qkv_b
    wqkv_eff[:, :E] *= qscale
    vqkv = vqkv.copy()
    vqkv[:E] *= qscale
    w1_eff = ln2_g[:, None] * w1
    vmlp = ln2_b @ w1 + b1

    has_qk_bias = bool(np.any(vqkv[: 2 * E] != 0.0))
    has_v_bias = bool(np.any(vqkv[2 * E:] != 0.0))
    has_out_bias = bool(np.any(out_b != 0.0))
    has_b2 = bool(np.any(b2 != 0.0))
    key = (has_qk_bias, has_v_bias, has_out_bias, has_b2)

    wqkv_c = (wqkv_eff * WS).astype(NPDT)
    wout_c = (out_w * WS).astype(NPDT)
    w1_c = (w1_eff * WS).astype(NPDT)
    w2_c = (w2 * WS).astype(NPDT)

    # masks: slot 0/2 = prev-block (keep q<=k), slot 1/3 = diag (keep k<=q)
    ki = np.arange(128)[:, None]
    qi = np.arange(128)[None, :]
    mprev = (qi <= ki).astype(np.float32)
    mdiag = (ki <= qi).astype(np.float32)
    msk = np.stack([mprev, mdiag, mprev, mdiag], axis=1)     # [128, 4, 128]
    msk = msk.reshape(128, 4 * 128).astype(ml_dtypes.bfloat16)

    # selbc: [8, HP, 128]: OS where h == 2*hp + (p>=64)
    selbc = np.zeros((8, HP, 128), np.float32)
    for hp in range(HP):
        selbc[2 * hp, hp, :64] = OS
        selbc[2 * hp + 1, hp, 64:] = OS
    selbc = selbc.reshape(8, HP * 128).astype(ml_dtypes.bfloat16)

    # x: transpose + halo + parity pack: [B, E, NP, U] -> flat [B, E, NT]
    xT_full = np.ascontiguousarray(x.transpose(0, 2, 1))  # [B, E, L]
    in_maps = []
    for core in range(N_CORES):
        s = core * S
        slab = np.zeros((B, E, S + HALO), np.float32)
        lo = s - HALO
        src_lo = max(lo, 0)
        slab[:, :, src_lo - lo:] = xT_full[:, :, src_lo:s + S]
        xpk = slab.reshape(B, E, U, NP).transpose(0, 1, 3, 2)  # [B, E, NP, U]
        xpk = np.ascontiguousarray(xpk).reshape(B, E, NT)

        # ohsel: [128, 2, H, H]: kind 0 = halo-valid one-hot, kind 1 = ones one-hot
        ohsel = np.zeros((128, 2, H, H), np.float32)
        halo_valid = 0.0 if core == 0 else 1.0
        for h in range(H):
            ohsel[:, 0, h, h] = halo_valid
            ohsel[:, 1, h, h] = 1.0
        ohsel = ohsel.reshape(128, 2 * H * H).astype(ml_dtypes.bfloat16)

        im = {
            "xp": xpk,
            "wqkv": wqkv_c,
            "wout": wout_c,
            "w1": w1_c,
            "w2": w2_c,
            "vmlp": vmlp.astype(np.float32),
            "msk": msk,
            "ohsel": ohsel,
            "selbc": selbc,
        }
        if has_qk_bias:
            im["vqk"] = vqkv[: 2 * E].astype(np.float32)
        if has_v_bias:
            im["vvb"] = vqkv[2 * E:].astype(np.float32)
            im["vhalo"] = np.full(128, halo_valid, np.float32)
        if has_out_bias:
            im["outb"] = out_b.astype(np.float32)
        if has_b2:
            im["b2v"] = b2.astype(np.float32)
        in_maps.append(im)
    return key, in_maps


_last_results = None


def kernel(**inputs) -> np.ndarray:
    global _last_results
    key, in_maps = _prepare_core_inputs(inputs)
    nc = _get_program(key)
    res = run_bass_kernel_spmd(nc, in_maps, core_ids=list(range(N_CORES)))
    _last_results = res
    out = np.empty((B, L, E), np.float32)
    for core in range(N_CORES):
        yp = res.results[core]["yT"]          # [B, E, SP] parity-packed
        yu = yp.reshape(B, E, NP, UQ).transpose(0, 1, 3, 2).reshape(B, E, S)
        out[:, core * S:(core + 1) * S, :] = yu.transpose(0, 2, 1)
    return out
